# revision 20
# baseline (speedup 1.0000x reference)
"""BiLSTM-CRF-Char kernel for 8 Trainium2 NeuronCores.

Strategy: data-parallel over batch B=32 -> 4 sentences/core.

v2 dispatch design (the baseline shipped 65MB of replicated weights and
re-jitted every call; wall time was ~2.6s/call of pure transport):
  - The Bass program + jitted shard_map executable are built ONCE and
    cached; subsequent kernel() calls reuse them (no re-trace/re-lower).
  - All weights (incl. the 100MB word-embedding table) are uploaded once
    as device-resident replicated jax arrays and reused across calls;
    a host-side np.array_equal check re-uploads only when they change.
  - Per-call data is a single packed (1, NPK) f32 row per core: char
    indices, word indices (as f32), and the CRF tag-dot mask. ~68KB/core.
  - Word embeddings are gathered ON DEVICE from the resident table via
    gpsimd indirect DMA + PE transpose (no 100MB host gather/upload).
  - Char embeddings never materialize: the (100, 64) char table is folded
    into the char-LSTM input weights on host (W_ce @ c_Wih^T -> (100,256))
    and applied to a device-built one-hot of the char indices.
  - Outputs are only (17,4) numerator partials + (17,4) CRF forward state
    per core; fetched with one batched device_get.
  - LSTM weights/states/one-hots, emissions, and the CRF scan run in
    bf16 (f32 PSUM accumulate): f32 matmuls lower to 2 PE passes each,
    so bf16 halves PE instruction count and speeds each pass.
  - Word-gate columns are host-permuted (i,f,g,o)->(i,f,o,g) so one
    sigmoid covers i,f,o per step; the input-projection term is seeded
    into PSUM by an identity matmul so no vector-add sits on the
    recurrence critical path. Per-call DMAs issue before weight-tile
    DMAs so the one-hot/gather pipeline starts immediately.
    Device exec 2.93ms -> 0.68ms; final rel err ~4e-5 (gate 2e-2).

Device (per core): char BiLSTM (batch 512, 16 steps), word BiLSTM (batch
4, 128 steps), emissions, numerator emission-dot, and CRF forward scan in
linear space:
    expA_{t+1} = (exp(trans - OFF).T @ expA_t) * exp(em_t) (masked blend)
with deterministic offset OFF=log(17) per step to avoid overflow (re-added
on host). The tags/transition part of the numerator and the final
logsumexp are O(S*B) host numpy ops.
"""

import sys

sys.path.insert(0, "/opt/trn_rl_repo")

import numpy as np
import ml_dtypes

import bass_rust
import jax
import concourse.bass as bass
import concourse.mybir as mybir
from concourse.tile import TileContext, ScopedClock
from concourse.bass2jax import (
    _bass_exec_p,
    partition_id_tensor,
    install_neuronx_cc_hook,
)
from jax.sharding import Mesh, PartitionSpec, NamedSharding
from jax.experimental.shard_map import shard_map


def _patched_drain_and_barrier(self, tick_clock, wait_clock):
    # This walrus build rejects instructions carrying many sync-waits
    # ("Too many sync wait commands"): split the kernel-tail drain's
    # waits into one NOP per semaphore wait.
    probe = self.nc.sync.nop()
    wait_clock.add_sem_waits(probe.ins, ScopedClock({None: tick_clock.global_clock}))
    si = probe.ins.sync_info
    waits = list(si.on_wait) if si is not None else []
    probe.ins.sync_info = (
        bass_rust.SyncInfo(on_wait=waits[:1], on_update=[]) if waits else None
    )
    for w in waits[1:]:
        n = self.nc.sync.nop()
        n.ins.sync_info = bass_rust.SyncInfo(on_wait=[w], on_update=[])
    self.nc.sync.drain()
    self.nc.all_engine_barrier()
    assert self.sems is not None
    popped = self.nc._tile_sem_poison_stack.pop()
    assert popped is self._sem_poison
    self.nc.clear_and_free_semaphores(list(self.sems.allocated().values()))
    self.nc.all_engine_barrier()


TileContext._drain_and_barrier = _patched_drain_and_barrier


def _split_sync_waits(nc, maxw=1):
    # Hoist excess per-instruction sync-waits onto same-engine NOPs
    # inserted just before (this walrus build caps waits per inst).
    k = 0
    for f in nc.m.functions:
        for bb in f.blocks:
            insts = list(bb.instructions)
            if not any(
                ins.sync_info is not None and len(ins.sync_info.on_wait) > maxw
                for ins in insts
            ):
                continue
            new = []
            for ins in insts:
                si = ins.sync_info
                if si is not None and len(si.on_wait) > maxw:
                    waits = list(si.on_wait)
                    head, tail = waits[: len(waits) - maxw], waits[len(waits) - maxw :]
                    for i in range(0, len(head), maxw):
                        n = bass_rust.InstNoOp(name=f"waitsplit_{k}")
                        k += 1
                        n.engine = ins.engine
                        n.sync_info = bass_rust.SyncInfo(
                            on_wait=head[i : i + maxw], on_update=[]
                        )
                        new.append(n)
                    ins.sync_info = bass_rust.SyncInfo(
                        on_wait=tail, on_update=list(si.on_update)
                    )
                new.append(ins)
            bb.instructions = new
    return k


S, B, C = 128, 32, 16
VW, VC = 100000, 100
DW, DC = 256, 64
HW, HC = 512, 128
Hw2, Hc2 = HW // 2, HC // 2  # 256, 64
T = 17
NCORE = 8
BL = B // NCORE  # 4 sentences per core
NCH = S * BL  # 512 char-batch per core
OFF = float(np.log(T))  # per-step CRF offset
# word-gate column permutation: torch order (i,f,g,o) -> (i,f,o,g) so the
# three sigmoid gates are contiguous and fuse into one ACT per step
_GPERM = np.concatenate(
    [np.arange(0, 512), np.arange(768, 1024), np.arange(512, 768)]
)

F32 = mybir.dt.float32
BF16 = mybir.dt.bfloat16
I32 = mybir.dt.int32
AF = mybir.ActivationFunctionType

# packed per-call layout (f32 offsets within the (1, NPK) row)
PK_CIDX = 0  # char indices, col = t*NCH + s*BL + b          [C*NCH]
PK_WIDX = C * NCH  # word indices, flat[p*4+j] = pos[j*128+p] [NCH]
PK_TAG = PK_WIDX + NCH  # tagdot (T, NCH) row-major           [T*NCH]
PK_ME = PK_TAG + T * NCH  # maskE (masked only)               [T*NCH]
PK_MI = PK_ME + T * NCH  # maskI (masked only)                [T*NCH]
NPK_PLAIN = PK_ME
NPK_MASKED = PK_MI + T * NCH

# weight (replicated, device-resident) input names in declare order
_WNAMES = (
    ["wtab", "ident", "ciota"]
    + ["cfoldT_" + d for d in "fb"]
    + sum(
        [
            ["cWhhT_" + d, "cb_" + d, "wWihT_" + d, "wWhhT_" + d, "wbT_" + d]
            for d in "fb"
        ],
        [],
    )
    + ["emit_WT", "emit_bT", "expT", "crf_start"]
)

_CTX = {}  # masked -> dispatch context dict
_DEVW = {}  # weight name -> device jax array
_RAWW = {}  # raw input key -> host np array (strong ref; identity + value cache)

# raw inputs that feed replicated device weights (order irrelevant)
_RAW_WEIGHT_KEYS = (
    ["W_we", "W_ce"]
    + sum(
        [["c_Wih_" + d, "c_Whh_" + d, "c_b_" + d] for d in "fb"],
        [],
    )
    + sum(
        [["w_Wih_" + d, "w_Whh_" + d, "w_b_" + d] for d in "fb"],
        [],
    )
    + ["emit_W", "emit_b", "crf_trans", "crf_start"]
)


def _build_nc(masked: bool):
    """Bass program (per core). Inputs: replicated weights + one packed
    per-call row. Outputs: numo (T,BL) numerator emission partials and
    expA_out (T,BL) final linear-space CRF forward state."""
    nc = bass.Bass()

    def inp(name, shape, dt=F32):
        return nc.declare_dram_parameter(name, list(shape), dt, isOutput=False)

    wtab = inp("wtab", (VW, DW))
    ident = inp("ident", (128, 128))
    ciota = inp("ciota", (VC, 1))
    cW = {}
    for d in ("f", "b"):
        cW["fold" + d] = inp("cfoldT_" + d, (VC, 4 * Hc2), BF16)
    for d in ("f", "b"):
        cW["hh" + d] = inp("cWhhT_" + d, (Hc2, 4 * Hc2), BF16)
        cW["b" + d] = inp("cb_" + d, (Hc2, 4))
        cW["wih" + d] = inp("wWihT_" + d, (HC + DW, 4 * Hw2), BF16)
        cW["whh" + d] = inp("wWhhT_" + d, (Hw2, 4 * Hw2), BF16)
        cW["wb" + d] = inp("wbT_" + d, (1, 4 * Hw2), BF16)
    emit_WT = inp("emit_WT", (HW, T), BF16)
    emit_bT = inp("emit_bT", (1, T), BF16)
    expT = inp("expT", (T, T), BF16)
    crf_start = inp("crf_start", (T, 1))
    npk = NPK_MASKED if masked else NPK_PLAIN
    percall = inp("percall", (1, npk))
    numo_out = nc.declare_dram_parameter("numo", [T, BL], F32, isOutput=True)
    expA_out = nc.declare_dram_parameter("expA_out", [T, BL], F32, isOutput=True)

    with TileContext(nc) as tc:
        with tc.tile_pool(name="persist", bufs=1) as pp:
            # ---- per-call + gather/one-hot inputs FIRST: they gate the
            # one-hot/gather pipeline start, so their DMAs must not queue
            # behind ~3MB of weight-tile loads ----
            t_ident = pp.tile([128, 128], F32)
            nc.sync.dma_start(out=t_ident[:, :], in_=ident[:, :])
            t_ident_bf = pp.tile([128, 128], BF16)
            nc.vector.tensor_copy(out=t_ident_bf[:, :], in_=t_ident[:, :])
            t_iota = pp.tile([VC, 1], F32)
            nc.sync.dma_start(out=t_iota[:, :], in_=ciota[:, :])
            t_widxf = pp.tile([128, BL], F32)
            nc.sync.dma_start(
                out=t_widxf[:, :], in_=percall[0:1, PK_WIDX : PK_WIDX + NCH]
            )
            t_widx = pp.tile([128, BL], I32)
            nc.vector.tensor_copy(out=t_widx[:, :], in_=t_widxf[:, :])
            t_tagdot = pp.tile([T, NCH], F32)
            nc.sync.dma_start(
                out=t_tagdot[:, :], in_=percall[0:1, PK_TAG : PK_TAG + T * NCH]
            )
            if masked:
                t_mE = pp.tile([T, NCH], F32)
                nc.sync.dma_start(
                    out=t_mE[:, :], in_=percall[0:1, PK_ME : PK_ME + T * NCH]
                )
                t_mI = pp.tile([T, NCH], F32)
                nc.sync.dma_start(
                    out=t_mI[:, :], in_=percall[0:1, PK_MI : PK_MI + T * NCH]
                )
            t_ones = pp.tile([1, NCH], F32)
            nc.vector.memset(t_ones[:, :], 1.0)
            t_ones_bf = pp.tile([1, NCH], BF16)
            nc.vector.memset(t_ones_bf[:, :], 1.0)
            t_ones_vc = pp.tile([1, VC], F32)
            nc.vector.memset(t_ones_vc[:, :], 1.0)

            # persistent state / activations
            t_oh = pp.tile([VC, C * NCH], BF16)  # char one-hot
            t_wek = [pp.tile([128, NCH], BF16, tag=f"wek{i}", name=f"wek{i}") for i in range(2)]
            t_hc = {d: pp.tile([Hc2, NCH], BF16, tag="hc" + d, name="hc" + d) for d in "fb"}
            t_cc = {d: pp.tile([Hc2, NCH], F32, tag="cc" + d, name="cc" + d) for d in "fb"}
            t_xch = pp.tile([128, NCH], BF16)  # char features, x k-chunk 0
            t_X = {d: pp.tile([128, 32 * S], BF16, tag="X" + d, name="X" + d) for d in "fb"}
            t_H = {
                (d, kc): pp.tile([128, 4 * S], BF16, tag=f"H{d}{kc}", name=f"H{d}{kc}")
                for d in "fb"
                for kc in range(2)
            }
            t_cw = pp.tile([128, 2 * BL * 2], F32)  # word c state [f(8) | b(8)]
            t_Eem = pp.tile([T, NCH], F32)
            t_expA = pp.tile([T, BL], BF16)
            t_expAf = pp.tile([T, BL], F32)

            # ================= char one-hot (device-built) =================
            with (
                tc.tile_pool(name="ohps", bufs=2, space="PSUM") as ohps,
                tc.tile_pool(name="ohwk", bufs=1) as ohwk,
            ):
                t_cidx = ohwk.tile([1, C * NCH], F32)
                nc.sync.dma_start(
                    out=t_cidx[:, :], in_=percall[0:1, PK_CIDX : PK_CIDX + C * NCH]
                )
                for o in range(4):
                    pb = ohps.tile([VC, 2048], F32, tag="pb", name="pb")
                    for q in range(4):
                        sl = slice(o * 2048 + q * 512, o * 2048 + (q + 1) * 512)
                        nc.tensor.matmul(
                            pb[:, q * 512 : (q + 1) * 512],
                            t_ones_vc[:, :], t_cidx[:, sl],
                            start=True, stop=True,
                        )
                    nc.vector.tensor_scalar(
                        out=t_oh[:, o * 2048 : (o + 1) * 2048], in0=pb[:, :],
                        scalar1=t_iota[:, 0:1], scalar2=None,
                        op0=mybir.AluOpType.is_equal,
                    )

            # ========== word embedding gather (indirect DMA + transpose) ==========
            with (
                tc.tile_pool(name="gtps", bufs=2, space="PSUM") as gtps,
                tc.tile_pool(name="gwk", bufs=2) as gwk,
            ):
                for j in range(4):
                    g = gwk.tile([128, DW], F32, tag="g", name="g")
                    nc.gpsimd.indirect_dma_start(
                        out=g[:, :], out_offset=None, in_=wtab[:, :],
                        in_offset=bass.IndirectOffsetOnAxis(
                            ap=t_widx[:, j : j + 1], axis=0
                        ),
                    )
                    for h in range(2):
                        pt = gtps.tile([128, 128], F32, tag="pt", name="pt")
                        nc.tensor.transpose(
                            out=pt[:, :], in_=g[:, h * 128 : (h + 1) * 128],
                            identity=t_ident[:, :],
                        )
                        nc.vector.tensor_copy(
                            out=t_wek[h][:, j * 128 : (j + 1) * 128], in_=pt[:, :]
                        )

            # ---- load persistent weights to SBUF ----
            tw = {}
            for d in ("f", "b"):
                tw["cfold" + d] = pp.tile(
                    [VC, 4 * Hc2], BF16, tag="cf" + d, name="cf" + d
                )
                nc.sync.dma_start(out=tw["cfold" + d][:, :], in_=cW["fold" + d][:, :])
                tw["chh" + d] = pp.tile([Hc2, 4 * Hc2], BF16, tag="chh" + d, name="chh" + d)
                nc.sync.dma_start(out=tw["chh" + d][:, :], in_=cW["hh" + d][:, :])
                tw["cb" + d] = pp.tile([Hc2, 4], F32, tag="cb" + d, name="cb" + d)
                nc.sync.dma_start(out=tw["cb" + d][:, :], in_=cW["b" + d][:, :])
                for kc in range(3):
                    t = pp.tile([128, 4 * Hw2], BF16, tag=f"wih{d}{kc}", name=f"wih{d}{kc}")
                    nc.sync.dma_start(
                        out=t[:, :], in_=cW["wih" + d][kc * 128 : (kc + 1) * 128, :]
                    )
                    tw[f"wih{d}{kc}"] = t
                for kc in range(2):
                    t = pp.tile([128, 4 * Hw2], BF16, tag=f"whh{d}{kc}", name=f"whh{d}{kc}")
                    nc.sync.dma_start(
                        out=t[:, :], in_=cW["whh" + d][kc * 128 : (kc + 1) * 128, :]
                    )
                    tw[f"whh{d}{kc}"] = t
                tw["wb" + d] = pp.tile([1, 4 * Hw2], BF16, tag="wb" + d, name="wb" + d)
                nc.sync.dma_start(out=tw["wb" + d][:, :], in_=cW["wb" + d][:, :])
            t_emitW = [pp.tile([128, T], BF16, tag=f"emw{k}", name=f"emw{k}") for k in range(4)]
            for kc in range(4):
                nc.sync.dma_start(
                    out=t_emitW[kc][:, :], in_=emit_WT[kc * 128 : (kc + 1) * 128, :]
                )
            t_emitb = pp.tile([1, T], BF16)
            nc.sync.dma_start(out=t_emitb[:, :], in_=emit_bT[:, :])
            t_expT = pp.tile([T, T], BF16)
            nc.sync.dma_start(out=t_expT[:, :], in_=expT[:, :])
            t_start = pp.tile([T, 1], F32)
            nc.sync.dma_start(out=t_start[:, :], in_=crf_start[:, :])

            # ================= char BiLSTM =================
            with (
                tc.tile_pool(name="cps", bufs=2, space="PSUM") as cps,
                tc.tile_pool(name="cwork", bufs=3) as cwk,
            ):
                for t in range(C):
                    ps_if = cps.tile([128, 2 * NCH], F32, tag="psif", name="psif")
                    ps_go = cps.tile([128, 2 * NCH], F32, tag="psgo", name="psgo")
                    for di, d in enumerate("fb"):
                        te = t if d == "f" else C - 1 - t
                        rx = t_oh[:, te * NCH : (te + 1) * NCH]
                        sl = slice(di * NCH, (di + 1) * NCH)
                        nc.tensor.matmul(
                            ps_if[:, sl], tw["cfold" + d][:, 0:128], rx,
                            start=True, stop=(t == 0),
                        )
                        nc.tensor.matmul(
                            ps_go[:, sl], tw["cfold" + d][:, 128:256], rx,
                            start=True, stop=(t == 0),
                        )
                        if t > 0:
                            nc.tensor.matmul(
                                ps_if[:, sl], tw["chh" + d][:, 0:128],
                                t_hc[d][:, :], start=False, stop=True,
                            )
                            nc.tensor.matmul(
                                ps_go[:, sl], tw["chh" + d][:, 128:256],
                                t_hc[d][:, :], start=False, stop=True,
                            )
                    for di, d in enumerate("fb"):
                        sl = slice(di * NCH, (di + 1) * NCH)
                        cb = tw["cb" + d]
                        si = cwk.tile([Hc2, NCH], F32, tag="si", name="si")
                        nc.scalar.activation(
                            si[:, :], ps_if[0:Hc2, sl], AF.Sigmoid, bias=cb[:, 0:1]
                        )
                        sf = cwk.tile([Hc2, NCH], F32, tag="sf", name="sf")
                        nc.scalar.activation(
                            sf[:, :], ps_if[Hc2:128, sl], AF.Sigmoid, bias=cb[:, 1:2]
                        )
                        tg = cwk.tile([Hc2, NCH], F32, tag="tg", name="tg")
                        nc.scalar.activation(
                            tg[:, :], ps_go[0:Hc2, sl], AF.Tanh, bias=cb[:, 2:3]
                        )
                        so = cwk.tile([Hc2, NCH], F32, tag="so", name="so")
                        nc.scalar.activation(
                            so[:, :], ps_go[Hc2:128, sl], AF.Sigmoid, bias=cb[:, 3:4]
                        )
                        if t == 0:
                            nc.vector.tensor_mul(
                                out=t_cc[d][:, :], in0=si[:, :], in1=tg[:, :]
                            )
                        else:
                            t1 = cwk.tile([Hc2, NCH], F32, tag="t1", name="t1")
                            nc.vector.tensor_mul(
                                out=t1[:, :], in0=si[:, :], in1=tg[:, :]
                            )
                            t2 = cwk.tile([Hc2, NCH], F32, tag="t2", name="t2")
                            nc.vector.tensor_mul(
                                out=t2[:, :], in0=sf[:, :], in1=t_cc[d][:, :]
                            )
                            nc.vector.tensor_add(
                                out=t_cc[d][:, :], in0=t1[:, :], in1=t2[:, :]
                            )
                        tcc = cwk.tile([Hc2, NCH], F32, tag="tcc", name="tcc")
                        nc.scalar.activation(tcc[:, :], t_cc[d][:, :], AF.Tanh)
                        nc.vector.tensor_mul(
                            out=t_hc[d][:, :], in0=so[:, :], in1=tcc[:, :]
                        )
                # assemble xT chunk0 = [h_f; h_b] (partition-moving: use DMA)
                nc.sync.dma_start(out=t_xch[0:Hc2, :], in_=t_hc["f"][:, :])
                nc.sync.dma_start(out=t_xch[Hc2:128, :], in_=t_hc["b"][:, :])

            # ================= word input projections =================
            xs = [t_xch, t_wek[0], t_wek[1]]
            with tc.tile_pool(name="wps", bufs=3, space="PSUM") as wps:
                for d in "fb":
                    Xap = t_X[d][:, :].rearrange(
                        "p (s m b) -> p s m b", s=S, m=8, b=BL
                    )
                    for m in range(8):
                        ps = wps.tile([128, NCH], F32, tag="psx", name="psx")
                        msl = slice(m * 128, (m + 1) * 128)
                        # kc=1,2 (word-embedding chunks) first: they don't
                        # depend on char outputs, so PE can run them while
                        # the ACT-bound char phase finishes; kc=0 (char
                        # features) accumulates last.
                        for j, kc in enumerate((1, 2, 0)):
                            nc.tensor.matmul(
                                ps[:, :], tw[f"wih{d}{kc}"][:, msl], xs[kc][:, :],
                                start=(j == 0), stop=False,
                            )
                        nc.tensor.matmul(
                            ps[:, :], tw["wb" + d][:, msl], t_ones_bf[:, :],
                            start=False, stop=True,
                        )
                        nc.vector.tensor_copy(
                            out=Xap[:, :, m, :],
                            in_=ps[:, :].rearrange("p (s b) -> p s b", s=S, b=BL),
                        )

            # ================= word BiLSTM recurrence =================
            with (
                tc.tile_pool(name="rps", bufs=2, space="PSUM") as rps,
                tc.tile_pool(name="rwork", bufs=3) as rwk,
            ):
                W8 = 8 * BL  # 32
                for sig in range(S):
                    sx = {"f": sig, "b": S - 1 - sig}
                    # gates accumulate fully in PSUM: the X (input-proj)
                    # term is seeded by an identity matmul, so no DVE
                    # add sits on the recurrence critical path and the
                    # activations read PSUM directly.
                    gs = rps.tile([128, 2 * W8], F32, tag="psg", name="psg")
                    for di, d in enumerate("fb"):
                        xsl = slice(sx[d] * W8, (sx[d] + 1) * W8)
                        hsl = slice(di * W8, (di + 1) * W8)
                        nc.tensor.matmul(
                            gs[:, hsl], t_ident_bf[:, :], t_X[d][:, xsl],
                            start=True, stop=(sig == 0),
                        )
                        if sig > 0:
                            sp = sx[d] - 1 if d == "f" else sx[d] + 1
                            for m in range(8):
                                msl = slice(m * 128, (m + 1) * 128)
                                for kc in range(2):
                                    rhs = t_H[(d, kc)][:, sp * BL : (sp + 1) * BL]
                                    nc.tensor.matmul(
                                        gs[:, di * W8 + m * BL : di * W8 + (m + 1) * BL],
                                        tw[f"whh{d}{kc}"][:, msl],
                                        rhs, start=False, stop=(kc == 1),
                                    )
                    # merged activations over both dirs; gs cols (d, m, b)
                    # with host-permuted gate order (i,f,o,g):
                    # i: [0:8], f: [8:16], o: [16:24], g: [24:32]
                    g3 = gs[:, :].rearrange("p (d x) -> p d x", d=2)
                    sif = rwk.tile([128, 48], F32, tag="wsif", name="wsif")
                    sif3 = sif[:, :].rearrange("p (d x) -> p d x", d=2)
                    nc.scalar.activation(sif3[:, :, :], g3[:, :, 0:24], AF.Sigmoid)
                    tg = rwk.tile([128, 16], F32, tag="wtg", name="wtg")
                    tg3 = tg[:, :].rearrange("p (d x) -> p d x", d=2)
                    nc.scalar.activation(tg3[:, :, :], g3[:, :, 24:32], AF.Tanh)
                    if sig == 0:
                        # c = sigmoid(i) * tanh(g)
                        cw3 = t_cw[:, :].rearrange("p (d x) -> p d x", d=2)
                        nc.vector.tensor_mul(
                            out=cw3[:, :, :], in0=sif3[:, :, 0:8], in1=tg3[:, :, :]
                        )
                    else:
                        cw3 = t_cw[:, :].rearrange("p (d x) -> p d x", d=2)
                        t1 = rwk.tile([128, 16], F32, tag="wt1", name="wt1")
                        t13 = t1[:, :].rearrange("p (d x) -> p d x", d=2)
                        nc.vector.tensor_mul(
                            out=t13[:, :, :], in0=sif3[:, :, 0:8], in1=tg3[:, :, :]
                        )
                        t2 = rwk.tile([128, 16], F32, tag="wt2", name="wt2")
                        t23 = t2[:, :].rearrange("p (d x) -> p d x", d=2)
                        nc.vector.tensor_mul(
                            out=t23[:, :, :], in0=sif3[:, :, 8:16], in1=cw3[:, :, :]
                        )
                        nc.vector.tensor_add(
                            out=t_cw[:, :], in0=t1[:, :], in1=t2[:, :]
                        )
                    tcw = rwk.tile([128, 16], F32, tag="wtc", name="wtc")
                    nc.scalar.activation(tcw[:, :], t_cw[:, :], AF.Tanh)
                    for di, d in enumerate("fb"):
                        for kc in range(2):
                            ocol = di * 24 + 16 + kc * BL
                            col = di * 8 + kc * BL
                            nc.vector.tensor_mul(
                                out=t_H[(d, kc)][:, sx[d] * BL : (sx[d] + 1) * BL],
                                in0=sif[:, ocol : ocol + BL],
                                in1=tcw[:, col : col + BL],
                            )

            # ================= emissions + CRF =================
            with tc.tile_pool(name="eps", bufs=1, space="PSUM") as eps:
                ps_em = eps.tile([T, NCH], F32)
                korder = [("f", 0), ("f", 1), ("b", 0), ("b", 1)]
                for i, (d, kc) in enumerate(korder):
                    Hap = t_H[(d, kc)][:, :]
                    nc.tensor.matmul(
                        ps_em[:, :], t_emitW[i][:, :], Hap,
                        start=(i == 0), stop=False,
                    )
                nc.tensor.matmul(
                    ps_em[:, :], t_emitb[:, :], t_ones_bf[:, :],
                    start=False, stop=True,
                )
                # numerator emission partials: numo[t,b] = sum_s em*tagdot
                t_prod = pp.tile([T, NCH], F32, tag="t_prod", name="t_prod")
                nc.vector.tensor_mul(
                    out=t_prod[:, :], in0=ps_em[:, :], in1=t_tagdot[:, :]
                )
                t_numo = pp.tile([T, BL], F32, tag="t_numo", name="t_numo")
                nc.vector.tensor_reduce(
                    out=t_numo[:, :],
                    in_=t_prod[:, :].rearrange("p (s b) -> p b s", s=S, b=BL),
                    axis=mybir.AxisListType.X,
                    op=mybir.AluOpType.add,
                )
                nc.sync.dma_start(out=numo_out[:, :], in_=t_numo[:, :])
                nc.scalar.activation(t_Eem[:, :], ps_em[:, :], AF.Exp)
                if masked:
                    nc.vector.tensor_mul(
                        out=t_Eem[:, :], in0=t_Eem[:, :], in1=t_mE[:, :]
                    )
                # init alpha: expA = exp(em_0 + start)
                nc.scalar.activation(
                    t_expA[:, :], ps_em[:, 0:BL], AF.Exp, bias=t_start[:, :]
                )
            with tc.tile_pool(name="cfps", bufs=2, space="PSUM") as cfps:
                for s in range(1, S):
                    psc = cfps.tile([T, BL], F32, tag="psc", name="psc")
                    nc.tensor.matmul(
                        psc[:, :], t_expT[:, :], t_expA[:, :], start=True, stop=True
                    )
                    esl = t_Eem[:, s * BL : (s + 1) * BL]
                    if masked:
                        ta = pp.tile([T, BL], F32, tag="cma", name="cma")
                        nc.vector.tensor_mul(out=ta[:, :], in0=psc[:, :], in1=esl)
                        tea = pp.tile([T, BL], F32, tag="cmc", name="cmc")
                        nc.vector.tensor_copy(out=tea[:, :], in_=t_expA[:, :])
                        tb = pp.tile([T, BL], F32, tag="cmb", name="cmb")
                        nc.vector.tensor_mul(
                            out=tb[:, :], in0=tea[:, :],
                            in1=t_mI[:, s * BL : (s + 1) * BL],
                        )
                        nc.vector.tensor_add(
                            out=t_expA[:, :], in0=ta[:, :], in1=tb[:, :]
                        )
                    else:
                        nc.vector.tensor_mul(
                            out=t_expA[:, :], in0=psc[:, :], in1=esl
                        )
                nc.vector.tensor_copy(out=t_expAf[:, :], in_=t_expA[:, :])
                nc.sync.dma_start(out=expA_out[:, :], in_=t_expAf[:, :])
    _split_sync_waits(nc, maxw=1)
    return nc


def _weights_host(inputs):
    """Host layout prep for all replicated weights."""
    f32 = np.float32
    W_we = np.asarray(inputs["W_we"], f32)
    W_ce = np.asarray(inputs["W_ce"], f32)
    w = {}
    w["wtab"] = np.ascontiguousarray(W_we)
    w["ident"] = np.eye(128, dtype=f32)
    w["ciota"] = np.arange(VC, dtype=f32).reshape(VC, 1)
    for d in ("f", "b"):
        cWih = np.asarray(inputs["c_Wih_" + d], f32)  # (4*Hc2, DC)
        w["cfoldT_" + d] = np.ascontiguousarray(W_ce @ cWih.T).astype(
            ml_dtypes.bfloat16
        )  # (VC, 4*Hc2)
        w["cWhhT_" + d] = np.ascontiguousarray(
            np.asarray(inputs["c_Whh_" + d], f32).T
        ).astype(ml_dtypes.bfloat16)
        cb = np.asarray(inputs["c_b_" + d], f32)
        w["cb_" + d] = np.ascontiguousarray(cb.reshape(4, Hc2).T)
        w["wWihT_" + d] = np.ascontiguousarray(
            np.asarray(inputs["w_Wih_" + d], f32).T[:, _GPERM]
        ).astype(ml_dtypes.bfloat16)
        w["wWhhT_" + d] = np.ascontiguousarray(
            np.asarray(inputs["w_Whh_" + d], f32).T[:, _GPERM]
        ).astype(ml_dtypes.bfloat16)
        w["wbT_" + d] = (
            np.asarray(inputs["w_b_" + d], f32)
            .reshape(1, -1)[:, _GPERM]
            .astype(ml_dtypes.bfloat16)
        )
    w["emit_WT"] = np.ascontiguousarray(
        np.asarray(inputs["emit_W"], f32).T
    ).astype(ml_dtypes.bfloat16)
    w["emit_bT"] = (
        np.asarray(inputs["emit_b"], f32).reshape(1, T).astype(ml_dtypes.bfloat16)
    )
    w["expT"] = np.exp(np.asarray(inputs["crf_trans"], f32) - OFF).astype(
        ml_dtypes.bfloat16
    )
    w["crf_start"] = np.asarray(inputs["crf_start"], f32).reshape(T, 1)
    return w


def _percall_host(inputs, masked, mask):
    """Build the packed (NCORE, NPK) per-call array."""
    f32 = np.float32
    sentence = np.asarray(inputs["sentence"])
    char = np.asarray(inputs["char"])
    tags = np.asarray(inputs["tags"])
    mf = mask.astype(f32)
    npk = NPK_MASKED if masked else NPK_PLAIN
    pk = np.empty((NCORE, npk), f32)
    cols = np.arange(NCH)
    for c in range(NCORE):
        bs = slice(c * BL, (c + 1) * BL)
        # char indices, col = t*NCH + s*BL + b
        pk[c, PK_CIDX : PK_CIDX + C * NCH] = (
            char[bs].transpose(2, 1, 0).reshape(-1).astype(f32)
        )
        # word indices packed so SBUF tile[p, j] = pos[j*128+p]
        flat_pos = sentence[:, bs].reshape(-1)  # col = s*BL+b
        pk[c, PK_WIDX : PK_WIDX + NCH] = (
            flat_pos.reshape(BL, 128).T.reshape(-1).astype(f32)
        )
        # tagdot (T, NCH): one-hot of tags weighted by step mask
        td = np.zeros((T, NCH), f32)
        trow = tags[:, bs].reshape(-1)  # col = s*BL+b
        wgt = mf[:, bs].copy()
        wgt[0] = 1.0
        td[trow, cols] = wgt.reshape(-1)
        pk[c, PK_TAG : PK_TAG + T * NCH] = td.reshape(-1)
        if masked:
            mk = mf[:, bs].reshape(NCH)
            pk[c, PK_ME : PK_ME + T * NCH] = np.broadcast_to(
                mk, (T, NCH)
            ).reshape(-1)
            pk[c, PK_MI : PK_MI + T * NCH] = np.broadcast_to(
                1.0 - mk, (T, NCH)
            ).reshape(-1)
    return pk


def _get_ctx(masked):
    if masked in _CTX:
        return _CTX[masked]
    install_neuronx_cc_hook()
    nc = _build_nc(masked)
    partition_name = nc.partition_id_tensor.name if nc.partition_id_tensor else None
    in_names, out_names, out_avals = [], [], []
    for alloc in nc.m.functions[0].allocations:
        if not isinstance(alloc, mybir.MemoryLocationSet):
            continue
        name = alloc.memorylocations[0].name
        if alloc.kind == "ExternalInput":
            if name != partition_name:
                in_names.append(name)
        elif alloc.kind == "ExternalOutput":
            out_names.append(name)
            out_avals.append(
                jax.core.ShapedArray(
                    tuple(alloc.tensor_shape), mybir.dt.np(alloc.dtype)
                )
            )
    all_in = list(in_names)
    if partition_name is not None:
        all_in.append(partition_name)

    def _body(*args):
        operands = list(args)
        if partition_name is not None:
            operands.append(partition_id_tensor())
        return tuple(
            _bass_exec_p.bind(
                *operands,
                out_avals=tuple(out_avals),
                in_names=tuple(all_in),
                out_names=tuple(out_names),
                lowering_input_output_aliases=(),
                sim_require_finite=True,
                sim_require_nnan=True,
                nc=nc,
            )
        )

    wset = set(_WNAMES)
    devices = jax.devices()[:NCORE]
    mesh = Mesh(np.asarray(devices), ("core",))
    in_specs = tuple(
        PartitionSpec() if n in wset else PartitionSpec("core") for n in in_names
    )
    sharded = jax.jit(
        shard_map(
            _body,
            mesh=mesh,
            in_specs=in_specs,
            out_specs=(PartitionSpec("core"),) * len(out_names),
            check_rep=False,
        ),
        keep_unused=True,
    )
    ctx = {
        "nc": nc,
        "sharded": sharded,
        "in_names": in_names,
        "out_names": out_names,
        "mesh": mesh,
        "repl_sh": NamedSharding(mesh, PartitionSpec()),
        "core_sh": NamedSharding(mesh, PartitionSpec("core")),
    }
    _CTX[masked] = ctx
    return ctx


def _upload_weights(ctx, inputs):
    """(Re)build all weight layouts and upload to device; cache raw refs.

    Host->device goes through the (slow) axon tunnel once per array via a
    single device, then replicates with a fast device-to-device broadcast
    (direct replicated device_put would pay the tunnel 8x: ~60s for the
    100MB word table vs ~3s this way)."""
    w = _weights_host(inputs)
    dev0 = ctx["mesh"].devices.flat[0]
    staged = {name: jax.device_put(w[name], dev0) for name in _WNAMES}
    for name in _WNAMES:
        _DEVW[name] = jax.device_put(staged[name], ctx["repl_sh"])
    for k in _RAW_WEIGHT_KEYS:
        _RAWW[k] = np.asarray(inputs[k])


def _raw_weights_equal(inputs):
    for k in _RAW_WEIGHT_KEYS:
        cached = _RAWW.get(k)
        if cached is None:
            return False
        cur = np.asarray(inputs[k])
        if cur is cached:
            continue  # identity short-circuit (strong ref held in _RAWW)
        if cached.shape != cur.shape or not np.array_equal(cached, cur):
            return False
    return True


def _launch(ctx, pk):
    args = [
        pk if n == "percall" else _DEVW[n] for n in ctx["in_names"]
    ]
    return ctx["sharded"](*args)


def _collect(ctx, outs):
    res = jax.device_get(list(outs))
    byname = dict(zip(ctx["out_names"], res))
    numo = byname["numo"].reshape(NCORE, T, BL)
    expA = byname["expA_out"].reshape(NCORE, T, BL)
    return numo, expA


def kernel(**inputs):
    sentence = np.asarray(inputs["sentence"])
    mask = sentence != 1  # (S, B)
    masked = not bool(mask.all())

    ctx = _get_ctx(masked)
    pk = _percall_host(inputs, masked, mask)
    outs = None
    if all(n in _DEVW for n in _WNAMES):
        # optimistic: dispatch with cached device weights, verify host-side
        # while the device runs, re-dispatch only if weights changed.
        outs = _launch(ctx, pk)
        if not _raw_weights_equal(inputs):
            _upload_weights(ctx, inputs)
            outs = _launch(ctx, pk)
    else:
        _upload_weights(ctx, inputs)
        outs = _launch(ctx, pk)

    # host-only numerator terms (tags/transitions), overlapped with device
    tags = np.asarray(inputs["tags"]).astype(np.int64)
    trans = np.asarray(inputs["crf_trans"], np.float64)
    start = np.asarray(inputs["crf_start"], np.float64)
    end = np.asarray(inputs["crf_end"], np.float64)
    mf = mask.astype(np.float64)  # (S, B)
    num = start[tags[0]].copy()
    prev = tags[0].copy()
    for s in range(1, S):
        num += trans[prev, tags[s]] * mf[s]
        prev = np.where(mf[s] > 0, tags[s], prev)
    num += end[prev]

    numo, expA = _collect(ctx, outs)
    # numerator: device emission partials + host tag/transition terms
    num = num + numo.astype(np.float64).sum(axis=1).reshape(B)  # order c*BL+b

    # denominator: alpha = log(expA) + OFF * n_steps
    expA_full = expA.astype(np.float64).transpose(1, 0, 2).reshape(T, B)
    nsteps = mf[1:].sum(axis=0)  # unmasked steps per sentence
    alpha = np.log(expA_full) + OFF * nsteps[None, :]
    av = alpha + end[:, None]
    amax = av.max(axis=0)
    den = amax + np.log(np.exp(av - amax).sum(axis=0))
    total = float((num - den).sum())
    return np.float32(-total)
